# revision 3
# baseline (speedup 1.0000x reference)
"""Trainium2 Bass kernel for nn_MultiHeadAttention (B=2, S=2048, D=1024, H=16).

Sharding: 8 cores = batch(2) x head-group(4); each core projects a 256-wide
(4-head) slice of Q/K/V for its batch (bf16 operands, fp32 accumulate),
computes transposed scores per head, exponentiates once on the Scalar engine
(softmax without max-subtraction; scores are O(1) so exp is safe), streams the
unnormalized exp'd scores to DRAM in k-major layout, and accumulates
ctx^T = [V|1]^T @ E^T (the ones column yields softmax denominators for free).
Context rows are normalized via a reciprocal row broadcast (DRAM-bounce DMA)
and fed to a K=128 out-projection producing a per-core partial output.
The host unshards: transposes + normalizes the attention matrix and sums the
output partials (+ bo).
"""
import numpy as np
import ml_dtypes
from contextlib import ExitStack

import concourse.bass as bass
import concourse.mybir as mybir
import concourse.tile as tile
from concourse import bacc
from concourse.bass_utils import run_bass_kernel_spmd

f32 = mybir.dt.float32
f32r = mybir.dt.float32r
bf16 = mybir.dt.bfloat16
AF = mybir.ActivationFunctionType

B, S, D, H, HD = 2, 2048, 1024, 16, 64
DCOL = 256
SCALE = HD ** -0.5

LAST_EXEC_NS = None
_TRACE = False


def build(S=S):
    ND = D // 128
    NSC = S // 512
    NST = S // 128

    nc = bacc.Bacc()
    xqT = nc.declare_dram_parameter("xqT", [D, S], bf16, isOutput=False)
    xkT = nc.declare_dram_parameter("xkT", [D, S], bf16, isOutput=False)
    xvT = nc.declare_dram_parameter("xvT", [D, S], bf16, isOutput=False)
    wq = nc.declare_dram_parameter("wq", [D, DCOL], bf16, isOutput=False)
    wk = nc.declare_dram_parameter("wk", [D, DCOL], bf16, isOutput=False)
    wv = nc.declare_dram_parameter("wv", [D, DCOL], bf16, isOutput=False)
    wo = nc.declare_dram_parameter("wo", [DCOL, D], bf16, isOutput=False)
    bq = nc.declare_dram_parameter("bq", [DCOL, 1], f32, isOutput=False)
    bk = nc.declare_dram_parameter("bk", [DCOL, 1], f32, isOutput=False)
    bv = nc.declare_dram_parameter("bv", [1, DCOL], bf16, isOutput=False)
    onesd = nc.declare_dram_parameter("onesd", [1, 128], bf16, isOutput=False)
    onesr = nc.declare_dram_parameter("onesr", [1, 128], f32, isOutput=False)
    et_out = nc.declare_dram_parameter("et_out", [S, 4, S], bf16, isOutput=True)
    outp = nc.declare_dram_parameter("outp", [S, D], f32, isOutput=True)
    rro = nc.dram_tensor("rro", [2 * (S // 512) * 2, 512], f32)

    with tile.TileContext(nc) as tc, ExitStack() as ctx:
        cst = ctx.enter_context(tc.tile_pool(name="cst", bufs=1))
        xtp = ctx.enter_context(tc.tile_pool(name="xtp", bufs=10))
        qkv = ctx.enter_context(tc.tile_pool(name="qkv", bufs=1))
        etp = ctx.enter_context(tc.tile_pool(name="etp", bufs=8))
        ctxp = ctx.enter_context(tc.tile_pool(name="ctxp", bufs=1))
        osbp = ctx.enter_context(tc.tile_pool(name="osbp", bufs=3))
        msb = ctx.enter_context(tc.tile_pool(name="msb", bufs=2))
        psb = ctx.enter_context(tc.tile_pool(name="psb", bufs=6, space="PSUM"))
        psc = ctx.enter_context(tc.tile_pool(name="psc", bufs=2, space="PSUM"))

        # ---- constants / weights ----
        w_sb = {}
        for name, drt in (("q", wq), ("k", wk), ("v", wv)):
            t_ = cst.tile([128, ND, DCOL], bf16, tag=f"w{name}", name=f"w{name}")
            nc.sync.dma_start(
                out=t_, in_=drt.rearrange("(d p) j -> p d j", p=128)
            )
            w_sb[name] = t_
        wo_sb2 = []
        for t2 in range(2):
            t_ = cst.tile([128, D], bf16, tag=f"wo{t2}", name=f"wo{t2}")
            nc.sync.dma_start(out=t_, in_=wo[t2 * 128:(t2 + 1) * 128, :])
            wo_sb2.append(t_)
        bq_sb = cst.tile([128, 2], f32, tag="bq")
        nc.sync.dma_start(out=bq_sb, in_=bq.rearrange("(t p) o -> p (t o)", p=128))
        bk_sb = cst.tile([128, 2], f32, tag="bk")
        nc.sync.dma_start(out=bk_sb, in_=bk.rearrange("(t p) o -> p (t o)", p=128))
        bvr_sb = cst.tile([1, DCOL], bf16, tag="bv")
        nc.sync.dma_start(out=bvr_sb, in_=bv[:, :])
        ones_sb = cst.tile([1, 128], bf16, tag="ones")
        nc.sync.dma_start(out=ones_sb, in_=onesd[:, :])
        ones_r = cst.tile([128, 128], f32r, tag="onesr")
        nc.sync.dma_start(out=ones_r[64:65, :], in_=onesr[:, :].bitcast(f32r))

        # ---- PE warm-up: keep HAM busy during the input-DMA ramp ----
        for wr in range(24):
            wps = psb.tile([128, 512], f32, tag="big", name="wps")
            nc.tensor.matmul(wps[0:64, 0:256], ones_sb[0:1, 0:64], bvr_sb,
                             start=True, stop=True)

        # ---- projections: QT / KT (transposed; bias per-partition) ----
        qt_sb = [qkv.tile([128, S], bf16, tag=f"qt{t}", name=f"qt{t}") for t in range(2)]
        kt_sb = [qkv.tile([128, S], bf16, tag=f"kt{t}", name=f"kt{t}") for t in range(2)]
        for xT, wsb, bias_sb, dest in (
            (xqT, w_sb["q"], bq_sb, qt_sb),
            (xkT, w_sb["k"], bk_sb, kt_sb),
        ):
            xts = []
            for d in range(ND):
                xt_ = xtp.tile([128, S], bf16, tag="xt", name="xt")
                nc.sync.dma_start(out=xt_, in_=xT[d * 128:(d + 1) * 128, :])
                xts.append(xt_)
            for sc in range(NSC):
                for t in range(2):
                    ps = psb.tile([128, 512], f32, tag="big", name="ps")
                    for d in range(ND):
                        nc.tensor.matmul(
                            ps, wsb[:, d, t * 128:(t + 1) * 128],
                            xts[d][:, sc * 512:(sc + 1) * 512],
                            start=(d == 0), stop=(d == ND - 1),
                        )
                    nc.scalar.activation(
                        out=dest[t][:, sc * 512:(sc + 1) * 512], in_=ps,
                        func=AF.Identity, bias=bias_sb[:, t:t + 1], scale=1.0,
                    )

        # ---- projections: V natural, ones column per head ----
        vt_sb = [qkv.tile([128, 4, 65], bf16, tag=f"vt{st}", name=f"vt{st}")
                 for st in range(NST)]
        for st in range(NST):
            nc.vector.memset(vt_sb[st][:, :, 64:65], 1.0)
        xts = []
        for d in range(ND):
            xt_ = xtp.tile([128, S], bf16, tag="xt", name="xt")
            nc.sync.dma_start(out=xt_, in_=xvT[d * 128:(d + 1) * 128, :])
            xts.append(xt_)
        for st in range(NST):
            ps = psb.tile([128, 512], f32, tag="big", name="ps")
            nc.tensor.matmul(ps[:, 0:DCOL], ones_sb, bvr_sb, start=True, stop=False)
            for d in range(ND):
                nc.tensor.matmul(
                    ps[:, 0:DCOL], xts[d][:, st * 128:(st + 1) * 128],
                    w_sb["v"][:, d, :], start=False, stop=(d == ND - 1),
                )
            nc.vector.tensor_copy(
                out=vt_sb[st][:, :, 0:64],
                in_=ps[:, 0:DCOL].rearrange("p (h e) -> p h e", h=4),
            )

        # ---- attention (qc outer; out-projection interleaved) ----
        ctxs = [ctxp.tile([128, S], bf16, tag=f"cx{t2}", name=f"cx{t2}") for t2 in range(2)]
        LOOK = 4
        for qc in range(NSC):
            for t in range(2):
                cps = [psc.tile([128, 512], f32, tag="ctx", name=f"cps{i_}")
                       for i_ in range(2)]
                ets = {}
                for kt in range(NST + LOOK):
                    if kt < NST:
                        etc = etp.tile([128, 2, 512], bf16, tag="et", name="etc")
                        spss = []
                        for i in range(2):
                            sps = psb.tile([128, 512], f32, tag="big", name="sps")
                            nc.tensor.matmul(
                                sps,
                                kt_sb[t][i * 64:(i + 1) * 64, kt * 128:(kt + 1) * 128],
                                qt_sb[t][i * 64:(i + 1) * 64, qc * 512:(qc + 1) * 512],
                                start=True, stop=True,
                            )
                            spss.append(sps)
                        for i in range(2):
                            nc.scalar.activation(out=etc[:, i, :], in_=spss[i], func=AF.Exp)
                        nc.sync.dma_start(
                            out=et_out[kt * 128:(kt + 1) * 128, 2 * t:2 * t + 2,
                                       qc * 512:(qc + 1) * 512],
                            in_=etc,
                        )
                        ets[kt] = etc
                    if kt >= LOOK:
                        pk = kt - LOOK
                        etc_p = ets.pop(pk)
                        for i in range(2):
                            nc.tensor.matmul(
                                cps[i][0:65, :], vt_sb[pk][:, 2 * t + i, :],
                                etc_p[:, i, :],
                                start=(pk == 0), stop=(pk == NST - 1),
                            )
                for i in range(2):
                    h = 2 * t + i
                    # evacuate ctx+denom row, then normalize off the PE path
                    ctxu = msb.tile([128, 512], f32, tag="ctxu", name="ctxu")
                    nc.vector.tensor_copy(out=ctxu[0:65, :], in_=cps[i][0:65, :])
                    rrow = msb.tile([128, 512], f32, tag="rrow", name="rrow")
                    nc.vector.reciprocal(out=rrow[64:65, :], in_=ctxu[64:65, :])
                    slot = (qc * 2 + t) * 2 + i
                    nc.sync.dma_start(out=rro[slot:slot + 1, :], in_=rrow[64:65, :])
                    rbs = msb.tile([64, 512], f32, tag="rbs", name="rbs")
                    rsrc = rro[slot:slot + 1, :]
                    nc.gpsimd.dma_start(
                        out=rbs,
                        in_=bass.AP(tensor=rsrc.tensor, offset=rsrc.offset,
                                    ap=[[0, 64]] + list(rsrc.ap)[1:]),
                    )
                    if i == 0:
                        nc.vector.tensor_mul(
                            ctxs[t][0:64, qc * 512:(qc + 1) * 512], ctxu[0:64, :], rbs
                        )
                    else:
                        cnt = msb.tile([64, 512], bf16, tag="cnt", name="cnt")
                        nc.vector.tensor_mul(cnt, ctxu[0:64, :], rbs)
                        nc.sync.dma_start(
                            out=ctxs[t][64:128, qc * 512:(qc + 1) * 512], in_=cnt
                        )
            # out projection, deferred two qc groups
            pqcs = [qc - 2] if qc >= 2 else []
            if qc == NSC - 1:
                pqcs += [qc - 1, qc]
            for pqc in pqcs:
                for stl in range(4):
                    st = pqc * 4 + stl
                    osb = osbp.tile([128, D], f32, tag="osb", name="osb")
                    for oc in range(2):
                        ps = psb.tile([128, 512], f32, tag="big", name="ps")
                        for t2 in range(2):
                            nc.tensor.matmul(
                                ps, ctxs[t2][:, st * 128:(st + 1) * 128],
                                wo_sb2[t2][:, oc * 512:(oc + 1) * 512],
                                start=(t2 == 0), stop=(t2 == 1),
                            )
                        nc.vector.tensor_copy(out=osb[:, oc * 512:(oc + 1) * 512], in_=ps)
                    nc.sync.dma_start(out=outp[st * 128:(st + 1) * 128, :], in_=osb)

    nc.finalize()
    return nc


def kernel(query, key, value, Wq, bq, Wk, bk, Wv, bv, Wo, bo):
    global LAST_EXEC_NS
    query, key, value = np.asarray(query), np.asarray(key), np.asarray(value)
    Wq, Wk, Wv, Wo = np.asarray(Wq), np.asarray(Wk), np.asarray(Wv), np.asarray(Wo)
    bq, bk, bv, bo = np.asarray(bq), np.asarray(bk), np.asarray(bv), np.asarray(bo)

    nc = build()
    bf = ml_dtypes.bfloat16
    ones = np.ones((1, 128), bf)
    in_maps = []
    for c in range(8):
        b, g = c // 4, c % 4
        sl = slice(DCOL * g, DCOL * g + DCOL)
        in_maps.append({
            "xqT": np.ascontiguousarray(query[b].T).astype(bf),
            "xkT": np.ascontiguousarray(key[b].T).astype(bf),
            "xvT": np.ascontiguousarray(value[b].T).astype(bf),
            "wq": np.ascontiguousarray(Wq[:, sl] * SCALE).astype(bf),
            "wk": np.ascontiguousarray(Wk[:, sl]).astype(bf),
            "wv": np.ascontiguousarray(Wv[:, sl]).astype(bf),
            "wo": np.ascontiguousarray(Wo[sl, :]).astype(bf),
            "bq": np.ascontiguousarray((bq[sl] * SCALE).reshape(DCOL, 1)),
            "bk": np.ascontiguousarray(bk[sl].reshape(DCOL, 1)),
            "bv": np.ascontiguousarray(bv[sl].reshape(1, DCOL)).astype(bf),
            "onesd": ones,
            "onesr": np.ones((1, 128), np.float32),
        })

    if _TRACE:
        import ntff_shim  # noqa: F401
    res = run_bass_kernel_spmd(nc, in_maps, list(range(8)), trace=_TRACE)
    LAST_EXEC_NS = res.exec_time_ns

    attn = np.empty((B, H, S, S), np.float32)
    output = np.zeros((B, S, D), np.float32)
    for c in range(8):
        b, g = c // 4, c % 4
        et = res.results[c]["et_out"].astype(np.float32)  # [S(k), 4, S(q)]
        den = et.sum(axis=0)                               # [4, S(q)]
        attn[b, 4 * g:4 * g + 4] = et.transpose(1, 2, 0) / den[:, :, None]
        output[b] += res.results[c]["outp"]
    output += bo
    return output, attn


# revision 4
# speedup vs baseline: 1.1638x; 1.1638x over previous
"""Trainium2 Bass kernel for nn_MultiHeadAttention (B=2, S=2048, D=1024, H=16).

Sharding: 8 cores = batch(2) x head-group(4); each core projects a 256-wide
(4-head) slice of Q/K/V for its batch (bf16 operands, fp32 accumulate),
computes transposed scores per head, exponentiates once on the Scalar engine
(softmax without max-subtraction; scores are O(1) so exp is safe), streams the
unnormalized exp'd scores to DRAM in k-major layout, and accumulates
ctx^T = [V|1]^T @ E^T (the ones column yields softmax denominators for free).
Context rows are normalized via a reciprocal row broadcast (DRAM-bounce DMA)
and fed to a K=128 out-projection producing a per-core partial output.
The host unshards: transposes + normalizes the attention matrix and sums the
output partials (+ bo).
"""
import numpy as np
import ml_dtypes
from contextlib import ExitStack

import concourse.bass as bass
import concourse.mybir as mybir
import concourse.tile as tile
from concourse import bacc
from concourse.bass_utils import run_bass_kernel_spmd

f32 = mybir.dt.float32
f32r = mybir.dt.float32r
bf16 = mybir.dt.bfloat16
AF = mybir.ActivationFunctionType

B, S, D, H, HD = 2, 2048, 1024, 16, 64
DCOL = 256
SCALE = HD ** -0.5

LAST_EXEC_NS = None
_TRACE = False


def build(S=S):
    ND = D // 128
    NSC = S // 512
    NST = S // 128

    nc = bacc.Bacc()
    xqT = nc.declare_dram_parameter("xqT", [D, S], bf16, isOutput=False)
    xkT = nc.declare_dram_parameter("xkT", [D, S], bf16, isOutput=False)
    xvT = nc.declare_dram_parameter("xvT", [D, S], bf16, isOutput=False)
    wq = nc.declare_dram_parameter("wq", [D, DCOL], bf16, isOutput=False)
    wk = nc.declare_dram_parameter("wk", [D, DCOL], bf16, isOutput=False)
    wv = nc.declare_dram_parameter("wv", [D, DCOL], bf16, isOutput=False)
    wo = nc.declare_dram_parameter("wo", [DCOL, D], bf16, isOutput=False)
    bq = nc.declare_dram_parameter("bq", [DCOL, 1], f32, isOutput=False)
    bk = nc.declare_dram_parameter("bk", [DCOL, 1], f32, isOutput=False)
    bv = nc.declare_dram_parameter("bv", [1, DCOL], bf16, isOutput=False)
    onesd = nc.declare_dram_parameter("onesd", [1, 128], bf16, isOutput=False)
    onesr = nc.declare_dram_parameter("onesr", [1, 128], f32, isOutput=False)
    et_out = nc.declare_dram_parameter("et_out", [S, 4, S], bf16, isOutput=True)
    outp = nc.declare_dram_parameter("outp", [S, D], f32, isOutput=True)
    rro = nc.dram_tensor("rro", [2 * (S // 512) * 2, 512], f32)

    with tile.TileContext(nc) as tc, ExitStack() as ctx:
        cst = ctx.enter_context(tc.tile_pool(name="cst", bufs=1))
        xtp = ctx.enter_context(tc.tile_pool(name="xtp", bufs=10))
        qkv = ctx.enter_context(tc.tile_pool(name="qkv", bufs=1))
        etp = ctx.enter_context(tc.tile_pool(name="etp", bufs=8))
        ctxp = ctx.enter_context(tc.tile_pool(name="ctxp", bufs=1))
        osbp = ctx.enter_context(tc.tile_pool(name="osbp", bufs=3))
        msb = ctx.enter_context(tc.tile_pool(name="msb", bufs=2))
        psb = ctx.enter_context(tc.tile_pool(name="psb", bufs=6, space="PSUM"))
        psc = ctx.enter_context(tc.tile_pool(name="psc", bufs=2, space="PSUM"))

        # ---- constants / weights ----
        w_sb = {}
        for name, drt in (("q", wq), ("k", wk), ("v", wv)):
            t_ = cst.tile([128, ND, DCOL], bf16, tag=f"w{name}", name=f"w{name}")
            nc.sync.dma_start(
                out=t_, in_=drt.rearrange("(d p) j -> p d j", p=128)
            )
            w_sb[name] = t_
        wo_sb2 = []
        for t2 in range(2):
            t_ = cst.tile([128, D], bf16, tag=f"wo{t2}", name=f"wo{t2}")
            nc.sync.dma_start(out=t_, in_=wo[t2 * 128:(t2 + 1) * 128, :])
            wo_sb2.append(t_)
        bq_sb = cst.tile([128, 2], f32, tag="bq")
        nc.sync.dma_start(out=bq_sb, in_=bq.rearrange("(t p) o -> p (t o)", p=128))
        bk_sb = cst.tile([128, 2], f32, tag="bk")
        nc.sync.dma_start(out=bk_sb, in_=bk.rearrange("(t p) o -> p (t o)", p=128))
        bvr_sb = cst.tile([1, DCOL], bf16, tag="bv")
        nc.sync.dma_start(out=bvr_sb, in_=bv[:, :])
        ones_sb = cst.tile([1, 128], bf16, tag="ones")
        nc.sync.dma_start(out=ones_sb, in_=onesd[:, :])
        ones_r = cst.tile([128, 128], f32r, tag="onesr")
        nc.sync.dma_start(out=ones_r[64:65, :], in_=onesr[:, :].bitcast(f32r))

        # ---- projections: QT / KT (transposed; bias per-partition) ----
        qt_sb = [qkv.tile([128, S], bf16, tag=f"qt{t}", name=f"qt{t}") for t in range(2)]
        kt_sb = [qkv.tile([128, S], bf16, tag=f"kt{t}", name=f"kt{t}") for t in range(2)]
        for xT, wsb, bias_sb, dest in (
            (xqT, w_sb["q"], bq_sb, qt_sb),
            (xkT, w_sb["k"], bk_sb, kt_sb),
        ):
            xts = []
            for d in range(ND):
                xt_ = xtp.tile([128, S], bf16, tag="xt", name="xt")
                nc.sync.dma_start(out=xt_, in_=xT[d * 128:(d + 1) * 128, :])
                xts.append(xt_)
            for sc in range(NSC):
                for t in range(2):
                    ps = psb.tile([128, 512], f32, tag="big", name="ps")
                    for d in range(ND):
                        nc.tensor.matmul(
                            ps, wsb[:, d, t * 128:(t + 1) * 128],
                            xts[d][:, sc * 512:(sc + 1) * 512],
                            start=(d == 0), stop=(d == ND - 1),
                        )
                    nc.scalar.activation(
                        out=dest[t][:, sc * 512:(sc + 1) * 512], in_=ps,
                        func=AF.Identity, bias=bias_sb[:, t:t + 1], scale=1.0,
                    )

        # ---- projections: V natural, ones column per head ----
        vt_sb = [qkv.tile([128, 4, 65], bf16, tag=f"vt{st}", name=f"vt{st}")
                 for st in range(NST)]
        for st in range(NST):
            nc.vector.memset(vt_sb[st][:, :, 64:65], 1.0)
        xts = []
        for d in range(ND):
            xt_ = xtp.tile([128, S], bf16, tag="xt", name="xt")
            nc.sync.dma_start(out=xt_, in_=xvT[d * 128:(d + 1) * 128, :])
            xts.append(xt_)
        for st in range(NST):
            ps = psb.tile([128, 512], f32, tag="big", name="ps")
            nc.tensor.matmul(ps[:, 0:DCOL], ones_sb, bvr_sb, start=True, stop=False)
            for d in range(ND):
                nc.tensor.matmul(
                    ps[:, 0:DCOL], xts[d][:, st * 128:(st + 1) * 128],
                    w_sb["v"][:, d, :], start=False, stop=(d == ND - 1),
                )
            nc.vector.tensor_copy(
                out=vt_sb[st][:, :, 0:64],
                in_=ps[:, 0:DCOL].rearrange("p (h e) -> p h e", h=4),
            )

        # ---- attention (qc outer; out-projection interleaved) ----
        ctxs = [ctxp.tile([128, S], bf16, tag=f"cx{t2}", name=f"cx{t2}") for t2 in range(2)]
        LOOK = 3
        for qc in range(NSC):
            for t in range(2):
                cps = [psc.tile([128, 512], f32, tag="ctx", name=f"cps{i_}")
                       for i_ in range(2)]
                ets = {}
                for kt in range(NST + LOOK):
                    if kt < NST:
                        etc = etp.tile([128, 2, 512], bf16, tag="et", name="etc")
                        spss = []
                        for i in range(2):
                            sps = psb.tile([128, 512], f32, tag="big", name="sps")
                            nc.tensor.matmul(
                                sps,
                                kt_sb[t][i * 64:(i + 1) * 64, kt * 128:(kt + 1) * 128],
                                qt_sb[t][i * 64:(i + 1) * 64, qc * 512:(qc + 1) * 512],
                                start=True, stop=True,
                            )
                            spss.append(sps)
                        for i in range(2):
                            nc.scalar.activation(out=etc[:, i, :], in_=spss[i], func=AF.Exp)
                        nc.sync.dma_start(
                            out=et_out[kt * 128:(kt + 1) * 128, 2 * t:2 * t + 2,
                                       qc * 512:(qc + 1) * 512],
                            in_=etc,
                        )
                        ets[kt] = etc
                    if kt >= LOOK:
                        pk = kt - LOOK
                        etc_p = ets.pop(pk)
                        for i in range(2):
                            nc.tensor.matmul(
                                cps[i][0:65, :], vt_sb[pk][:, 2 * t + i, :],
                                etc_p[:, i, :],
                                start=(pk == 0), stop=(pk == NST - 1),
                            )
                for i in range(2):
                    h = 2 * t + i
                    # evacuate ctx+denom row, then normalize off the PE path
                    ctxu = msb.tile([128, 512], f32, tag="ctxu", name="ctxu")
                    nc.vector.tensor_copy(out=ctxu[0:65, :], in_=cps[i][0:65, :])
                    rrow = msb.tile([128, 512], f32, tag="rrow", name="rrow")
                    nc.vector.reciprocal(out=rrow[64:65, :], in_=ctxu[64:65, :])
                    slot = (qc * 2 + t) * 2 + i
                    nc.sync.dma_start(out=rro[slot:slot + 1, :], in_=rrow[64:65, :])
                    rbs = msb.tile([64, 512], f32, tag="rbs", name="rbs")
                    rsrc = rro[slot:slot + 1, :]
                    nc.gpsimd.dma_start(
                        out=rbs,
                        in_=bass.AP(tensor=rsrc.tensor, offset=rsrc.offset,
                                    ap=[[0, 64]] + list(rsrc.ap)[1:]),
                    )
                    if i == 0:
                        nc.vector.tensor_mul(
                            ctxs[t][0:64, qc * 512:(qc + 1) * 512], ctxu[0:64, :], rbs
                        )
                    else:
                        cnt = msb.tile([64, 512], bf16, tag="cnt", name="cnt")
                        nc.vector.tensor_mul(cnt, ctxu[0:64, :], rbs)
                        nc.sync.dma_start(
                            out=ctxs[t][64:128, qc * 512:(qc + 1) * 512], in_=cnt
                        )
            # out projection, deferred two qc groups
            pqcs = [qc - 2] if qc >= 2 else []
            if qc == NSC - 1:
                pqcs += [qc - 1, qc]
            for pqc in pqcs:
                for stl in range(4):
                    st = pqc * 4 + stl
                    osb = osbp.tile([128, D], f32, tag="osb", name="osb")
                    for oc in range(2):
                        ps = psb.tile([128, 512], f32, tag="big", name="ps")
                        for t2 in range(2):
                            nc.tensor.matmul(
                                ps, ctxs[t2][:, st * 128:(st + 1) * 128],
                                wo_sb2[t2][:, oc * 512:(oc + 1) * 512],
                                start=(t2 == 0), stop=(t2 == 1),
                            )
                        nc.vector.tensor_copy(out=osb[:, oc * 512:(oc + 1) * 512], in_=ps)
                    nc.sync.dma_start(out=outp[st * 128:(st + 1) * 128, :], in_=osb)

    nc.finalize()
    return nc


def kernel(query, key, value, Wq, bq, Wk, bk, Wv, bv, Wo, bo):
    global LAST_EXEC_NS
    query, key, value = np.asarray(query), np.asarray(key), np.asarray(value)
    Wq, Wk, Wv, Wo = np.asarray(Wq), np.asarray(Wk), np.asarray(Wv), np.asarray(Wo)
    bq, bk, bv, bo = np.asarray(bq), np.asarray(bk), np.asarray(bv), np.asarray(bo)

    nc = build()
    bf = ml_dtypes.bfloat16
    ones = np.ones((1, 128), bf)
    in_maps = []
    for c in range(8):
        b, g = c // 4, c % 4
        sl = slice(DCOL * g, DCOL * g + DCOL)
        in_maps.append({
            "xqT": np.ascontiguousarray(query[b].T).astype(bf),
            "xkT": np.ascontiguousarray(key[b].T).astype(bf),
            "xvT": np.ascontiguousarray(value[b].T).astype(bf),
            "wq": np.ascontiguousarray(Wq[:, sl] * SCALE).astype(bf),
            "wk": np.ascontiguousarray(Wk[:, sl]).astype(bf),
            "wv": np.ascontiguousarray(Wv[:, sl]).astype(bf),
            "wo": np.ascontiguousarray(Wo[sl, :]).astype(bf),
            "bq": np.ascontiguousarray((bq[sl] * SCALE).reshape(DCOL, 1)),
            "bk": np.ascontiguousarray(bk[sl].reshape(DCOL, 1)),
            "bv": np.ascontiguousarray(bv[sl].reshape(1, DCOL)).astype(bf),
            "onesd": ones,
            "onesr": np.ones((1, 128), np.float32),
        })

    if _TRACE:
        import ntff_shim  # noqa: F401
    res = run_bass_kernel_spmd(nc, in_maps, list(range(8)), trace=_TRACE)
    LAST_EXEC_NS = res.exec_time_ns

    attn = np.empty((B, H, S, S), np.float32)
    output = np.zeros((B, S, D), np.float32)
    for c in range(8):
        b, g = c // 4, c % 4
        et = res.results[c]["et_out"].astype(np.float32)  # [S(k), 4, S(q)]
        den = et.sum(axis=0)                               # [4, S(q)]
        attn[b, 4 * g:4 * g + 4] = et.transpose(1, 2, 0) / den[:, :, None]
        output[b] += res.results[c]["outp"]
    output += bo
    return output, attn
